# revision 1
# baseline (speedup 1.0000x reference)
"""FLAME forward (pose -> LBS) as a Bass/Tile kernel on 8 trn2 NeuronCores.

Strategy (pure data parallelism, batch sharded 8 x 128):
  Host (tiny math, O(B*J)):
    - rot6d / rodrigues -> rotation matrices, kinematic chain -> A[B,5,3,4]
    - pose_feat[B,36]
  Device (per core, partition dim = 128 batches):
    - pose_bs = PF^T @ posedirs_rhs           (PE, K=36)
    - v = vs + pose_bs                        (DVE)
    - T_hw[b,v] = sum_j A[b,j,h,w] w[v,j]     (PE, K=5, 12 maps)
    - out_h = sum_w T_hw * v_w + T_h3         (DVE elementwise)
"""

import numpy as np
from contextlib import ExitStack

B, V, J, P = 1024, 5023, 5, 36
NCORES = 8
BC = B // NCORES  # 128 batches per core = partition dim
PARENTS = np.array([0, 0, 1, 1, 1], dtype=np.int64)

# ---------------------------------------------------------------- host math


def _rodrigues(rv, eps=1e-8):
    # rv: [N,3] -> [N,3,3]
    ang = np.linalg.norm(rv + eps, axis=1, keepdims=True)  # [N,1]
    d = rv / ang
    cos = np.cos(ang)[:, :, None]
    sin = np.sin(ang)[:, :, None]
    rx, ry, rz = d[:, 0], d[:, 1], d[:, 2]
    z = np.zeros_like(rx)
    K = np.stack([z, -rz, ry, rz, z, -rx, -ry, rx, z], axis=1).reshape(-1, 3, 3)
    I = np.eye(3, dtype=rv.dtype)[None]
    return I + sin * K + (1.0 - cos) * (K @ K)


def _rot6d(x):
    a1, a2 = x[:, :3], x[:, 3:]
    b1 = a1 / np.linalg.norm(a1, axis=-1, keepdims=True)
    b2 = a2 - np.sum(b1 * a2, axis=-1, keepdims=True) * b1
    b2 = b2 / np.linalg.norm(b2, axis=-1, keepdims=True)
    b3 = np.cross(b1, b2)
    return np.stack([b1, b2, b3], axis=-2)


def _make_T(R, t):
    # R [...,3,3], t [...,3] -> [...,4,4]
    top = np.concatenate([R, t[..., None]], axis=-1)
    bot = np.broadcast_to(
        np.array([0.0, 0.0, 0.0, 1.0], R.dtype), top.shape[:-2] + (1, 4)
    )
    return np.concatenate([top, bot], axis=-2)


def host_prep(inputs):
    """Small-tensor math -> (A34 [B,5,3,4], PF [B,36]) in float32."""
    g6 = np.asarray(inputs["global_pose_params_6d"], np.float64)
    nk = np.asarray(inputs["neck_pose_params_ax"], np.float64)
    jw = np.asarray(inputs["jaw_pose_params_ax"], np.float64)
    ey = np.asarray(inputs["eye_pose_params_ax"], np.float64)
    jt = np.asarray(inputs["J_transformed_rest"], np.float64)  # [B,5,3]

    Rg = _rot6d(g6)
    Rn = _rodrigues(nk)
    Rj = _rodrigues(jw)
    Rel = _rodrigues(ey[:, :3])
    Rer = _rodrigues(ey[:, 3:])
    rot_mats = np.stack([Rg, Rn, Rj, Rel, Rer], axis=1)  # [B,5,3,3]

    rel = jt.copy()
    rel[:, 1:] -= jt[:, PARENTS[1:]]
    Tm = _make_T(rot_mats, rel)  # [B,5,4,4]
    chain = [Tm[:, 0]]
    for i in range(1, J):
        chain.append(chain[int(PARENTS[i])] @ Tm[:, i])
    tr = np.stack(chain, axis=1)  # [B,5,4,4]
    posed = tr[:, :, :3, 3]
    Rw = tr[:, :, :3, :3]
    t = posed - np.einsum("bjhw,bjw->bjh", Rw, jt)
    A = _make_T(Rw, t)  # [B,5,4,4]

    A34 = np.ascontiguousarray(A[:, :, :3, :4], np.float32)
    PF = np.ascontiguousarray(
        (rot_mats[:, 1:5] - np.eye(3)).reshape(B, -1), np.float32
    )
    return A34, PF


def host_reference_emulation(inputs):
    """Numpy emulation of exactly what the device computes (for validation)."""
    A34, PF = host_prep(inputs)
    vs = np.asarray(inputs["v_shaped_expressed"], np.float32).reshape(B, V * 3)
    W = np.asarray(inputs["lbs_weights"], np.float32)  # [V,5]
    pd = np.asarray(inputs["posedirs"], np.float32)  # [V,36,3]
    PDt = pd.transpose(1, 0, 2).reshape(36, V * 3)
    pbs = PF @ PDt  # [B, V*3]
    v = (vs + pbs).reshape(B, V, 3)
    T = np.einsum("bjhw,vj->bvhw", A34, W)  # [B,V,3,4]
    out = np.einsum("bvhw,bvw->bvh", T[:, :, :, :3], v) + T[:, :, :, 3]
    return out.astype(np.float32)


# ---------------------------------------------------------------- bass build

SLAB = 1024  # vertices per DMA slab
PAD = 8  # spare columns so f32r even-N padding never reads out of range
CH = 256  # vertices per compute chunk
NMAX = 512  # max matmul free dim (fp32)


def build_nc(bc=BC, v=V):
    import concourse.bacc as bacc
    import concourse.bass as bass_mod
    import concourse.tile as tile
    from concourse import mybir

    f32 = mybir.dt.float32
    f32r = mybir.dt.float32r

    # Bacc (not plain Bass): its finalize() runs generate_event_semaphores,
    # which splits multi-wait instructions to satisfy the TRN2 1-wait limit.
    nc = bacc.Bacc()
    vs_d = nc.dram_tensor("vs", [bc, v * 3], f32, kind="ExternalInput")
    # wat = [Wt | AT]: lbs_weights^T and the A-matrix lhsT columns share one
    # tensor (and one DMA semaphore) because one matmul reads both.
    wat_d = nc.dram_tensor("wat", [5, v + PAD + 12 * bc], f32r, kind="ExternalInput")
    # pfpd = [PFt | PDt]: pose-feature lhsT + posedirs rhs, same reason.
    pfpd_d = nc.dram_tensor("pfpd", [36, bc + v * 3 + PAD], f32r, kind="ExternalInput")
    out_d = nc.dram_tensor("out", [bc, v * 3], f32, kind="ExternalOutput")

    with tile.TileContext(nc) as tc, ExitStack() as ctx:
        singles = ctx.enter_context(tc.tile_pool(name="singles", bufs=1))
        sb_wat = singles.tile([5, v + PAD + 12 * bc], f32r)
        nc.sync.dma_start(out=sb_wat, in_=wat_d[:])
        sb_pfpd = singles.tile([36, bc + v * 3 + PAD], f32r)
        nc.sync.dma_start(out=sb_pfpd, in_=pfpd_d[:])
        sb_pf = sb_pfpd[:, :bc]

        vs_pool = ctx.enter_context(tc.tile_pool(name="vsp", bufs=2))
        out_pool = ctx.enter_context(tc.tile_pool(name="outp", bufs=2))
        t_pool = ctx.enter_context(tc.tile_pool(name="tsb", bufs=3))
        v_pool = ctx.enter_context(tc.tile_pool(name="vv", bufs=3))
        m_pool = ctx.enter_context(tc.tile_pool(name="mm", bufs=4))
        ppbs = ctx.enter_context(tc.tile_pool(name="ppbs", bufs=2, space="PSUM"))
        pT = ctx.enter_context(tc.tile_pool(name="pT", bufs=2, space="PSUM"))

        for s0 in range(0, v, SLAB):
            sv = min(SLAB, v - s0)
            vs_t = vs_pool.tile([bc, sv * 3], f32, tag="vs")
            nc.sync.dma_start(out=vs_t, in_=vs_d[:, s0 * 3 : (s0 + sv) * 3])
            out_t = out_pool.tile([bc, sv * 3], f32, tag="out")
            out3 = out_t[:].rearrange("p (a c) -> p a c", c=3)

            for c0 in range(s0, s0 + sv, CH):
                cv = min(CH, s0 + sv - c0)
                co = c0 - s0  # offset within slab

                # pose blendshapes for this chunk: [bc, cv*3] in PSUM
                # (fixed CH-sized alloc keeps matmul targets bank-aligned)
                pbs_full = ppbs.tile([bc, CH * 3], f32, tag="pbs")
                pbs = pbs_full[:, : cv * 3]
                for n0 in range(0, cv * 3, NMAX):
                    nn = min(NMAX, cv * 3 - n0)
                    nn += nn & 1  # f32r needs even moving dim
                    nc.tensor.matmul(
                        pbs_full[:, n0 : n0 + nn],
                        lhsT=sb_pf,
                        rhs=sb_pfpd[
                            :, bc + c0 * 3 + n0 : bc + c0 * 3 + n0 + nn
                        ],
                        start=True,
                        stop=True,
                    )

                # v = vs + pbs  [bc, cv, 3]
                v_t = v_pool.tile([bc, cv * 3], f32, tag="v")
                nc.vector.tensor_add(
                    v_t[:], vs_t[:, co * 3 : (co + cv) * 3], pbs[:]
                )
                v3 = v_t[:].rearrange("p (a c) -> p a c", c=3)

                for h in range(3):
                    # T maps for this h: [bc, 4, CH] in PSUM (w-planes bank-aligned)
                    Tp = pT.tile([bc, 4, CH], f32, tag="T")
                    for w in range(4):
                        hw = h * 4 + w
                        cvp = cv + (cv & 1)
                        nc.tensor.matmul(
                            Tp[:, w, :cvp],
                            lhsT=sb_wat[:, v + PAD + hw * bc : v + PAD + (hw + 1) * bc],
                            rhs=sb_wat[:, c0 : c0 + cvp],
                            start=True,
                            stop=True,
                        )
                    T_sb = t_pool.tile([bc, 4, cv], f32, tag="tsb")
                    nc.scalar.copy(T_sb[:], Tp[:, :, :cv])

                    m = m_pool.tile([bc, 3, cv], f32, tag="m")
                    vt_ap = v_t[:]
                    vb = bass_mod.AP(
                        tensor=vt_ap.tensor,
                        offset=vt_ap.offset,
                        ap=[list(vt_ap.ap[0]), [1, 3], [3, cv]],
                    )
                    nc.vector.tensor_tensor(
                        m[:], T_sb[:, :3, :], vb, op=mybir.AluOpType.mult
                    )
                    s01 = m_pool.tile([bc, cv], f32, tag="s01")
                    s2 = m_pool.tile([bc, cv], f32, tag="s2")
                    nc.vector.tensor_add(s01[:], m[:, 0, :], m[:, 1, :])
                    nc.vector.tensor_add(s2[:], s01[:], m[:, 2, :])
                    nc.vector.tensor_add(
                        out3[:, co : co + cv, h], s2[:], T_sb[:, 3, :]
                    )

            nc.sync.dma_start(out=out_d[:, s0 * 3 : (s0 + sv) * 3], in_=out_t[:])

    _strip_matmul_self_waits(nc)
    if not nc.is_finalized():
        nc.finalize()  # Bacc.compile(): reg alloc + wait splitting
    return nc


def _strip_matmul_self_waits(nc):
    """Drop redundant same-engine self-waits from Matmult instructions.

    Tile emits pool-slot release waits for every accessor proc, including the
    PE itself. With a fully unrolled kernel the PE queue executes in order, so
    a PE instruction waiting on the PE tick semaphore is always already
    satisfied — but walrus codegen only has one sync-wait slot for LDWEIGHTS,
    so a matmul carrying [other-engine wait, PE self-wait] fails to compile.
    """
    fn = nc.m.functions[0]
    # Own tick semaphores: the sems PE instructions themselves increment.
    pe_sems = set()
    for b in fn.blocks:
        for i in b.instructions:
            if i.opcode == "Matmult":
                for u in i.sync_info.on_update:
                    if u.ant_name.startswith("PE"):
                        pe_sems.add(u.ant_name)
    for b in fn.blocks:
        for i in b.instructions:
            if i.opcode != "Matmult":
                continue
            si = i.sync_info
            kept = [w for w in si.on_wait if w.ant_name not in pe_sems]
            if len(kept) != len(si.on_wait):
                si.on_wait = kept
                i.sync_info = si


# ---------------------------------------------------------------- entry point

_BUILT = {}


def _get_nc():
    if "nc" not in _BUILT:
        _BUILT["nc"] = build_nc()
    return _BUILT["nc"]


def make_in_maps(inputs):
    A34, PF = host_prep(inputs)
    vs = np.ascontiguousarray(
        np.asarray(inputs["v_shaped_expressed"], np.float32).reshape(B, V * 3)
    )
    W = np.asarray(inputs["lbs_weights"], np.float32)
    pd = np.asarray(inputs["posedirs"], np.float32)
    Wt = np.ascontiguousarray(W.T)  # [5, V]
    PDt = np.ascontiguousarray(pd.transpose(1, 0, 2).reshape(36, V * 3))
    PFt = np.ascontiguousarray(PF.T)  # [36, B]

    in_maps = []
    for c in range(NCORES):
        sl = slice(c * BC, (c + 1) * BC)
        # AT[j, (h*4+w)*BC + b] = A34[b, j, h, w] for this core's batches
        AT_c = A34[sl].transpose(1, 2, 3, 0).reshape(5, 12 * BC)
        pad5 = np.zeros((5, PAD), np.float32)
        pad36 = np.zeros((36, PAD), np.float32)
        wat = np.ascontiguousarray(np.concatenate([Wt, pad5, AT_c], axis=1))
        pfpd = np.ascontiguousarray(
            np.concatenate([PFt[:, sl], PDt, pad36], axis=1)
        )
        in_maps.append(
            {
                "vs": np.ascontiguousarray(vs[sl]),
                "wat": wat,
                "pfpd": pfpd,
            }
        )
    return in_maps


def run_on_device(inputs, trace=False):
    from concourse.bass_utils import run_bass_kernel_spmd

    nc = _get_nc()
    in_maps = make_in_maps(inputs)
    res = run_bass_kernel_spmd(nc, in_maps, list(range(NCORES)), trace=trace)
    out = np.concatenate([res.results[i]["out"] for i in range(NCORES)], axis=0)
    return out.reshape(B, V, 3).astype(np.float32), res


def kernel(**inputs):
    out, _ = run_on_device(inputs, trace=False)
    return out



# revision 3
# speedup vs baseline: 1.4862x; 1.4862x over previous
"""FLAME forward (pose -> LBS) as a Bass/Tile kernel on 8 trn2 NeuronCores.

Strategy (pure data parallelism, batch sharded 8 x 128, fp16 on device):
  Host (tiny math, O(B*J)):
    - rot6d / rodrigues -> rotation matrices, kinematic chain -> A[B,5,3,4]
    - pose_feat[B,36]
  Device (per core, partition dim = 128 batches, vertices padded to 5120):
    - v = vs + PF @ posedirs   accumulated in PSUM: pose-blendshape matmul
      (K=36) plus an identity matmul (K=128) that adds vs, both fp16 -> f32
    - T[h] = blockdiag(W) @ A  (K=10 w-pair block-diagonal trick: one matmul
      yields two w-planes, 6 matmuls per chunk instead of 12)
    - drains: Act copies T psum->sbuf fp16, DVE copies v
    - combine out_h = sum_w T_hw*v_w + T_h3 on DVE (fp16 2x mode) + GpSimd
  Layouts are w-major ([.., 3, V]) so every DVE op is stride-1 packed fp16.
"""

import numpy as np
from contextlib import ExitStack

B, V, J, P = 1024, 5023, 5, 36
NCORES = 8
BC = B // NCORES  # 128 batches per core = partition dim
PARENTS = np.array([0, 0, 1, 1, 1], dtype=np.int64)

VP = 5120  # V padded to a multiple of CH
CH = 256  # vertices per compute chunk
NCHUNK = VP // CH
VS_SLAB = 1024  # vertices per vs/out DMA slab
F16 = np.float16

# ---------------------------------------------------------------- host math


def _rodrigues(rv, eps=1e-8):
    # rv: [N,3] -> [N,3,3]
    ang = np.linalg.norm(rv + eps, axis=1, keepdims=True)  # [N,1]
    d = rv / ang
    cos = np.cos(ang)[:, :, None]
    sin = np.sin(ang)[:, :, None]
    rx, ry, rz = d[:, 0], d[:, 1], d[:, 2]
    z = np.zeros_like(rx)
    K = np.stack([z, -rz, ry, rz, z, -rx, -ry, rx, z], axis=1).reshape(-1, 3, 3)
    I = np.eye(3, dtype=rv.dtype)[None]
    return I + sin * K + (1.0 - cos) * (K @ K)


def _rot6d(x):
    a1, a2 = x[:, :3], x[:, 3:]
    b1 = a1 / np.linalg.norm(a1, axis=-1, keepdims=True)
    b2 = a2 - np.sum(b1 * a2, axis=-1, keepdims=True) * b1
    b2 = b2 / np.linalg.norm(b2, axis=-1, keepdims=True)
    b3 = np.cross(b1, b2)
    return np.stack([b1, b2, b3], axis=-2)


def _make_T(R, t):
    # R [...,3,3], t [...,3] -> [...,4,4]
    top = np.concatenate([R, t[..., None]], axis=-1)
    bot = np.broadcast_to(
        np.array([0.0, 0.0, 0.0, 1.0], R.dtype), top.shape[:-2] + (1, 4)
    )
    return np.concatenate([top, bot], axis=-2)


def host_prep(inputs):
    """Small-tensor math -> (A34 [B,5,3,4], PF [B,36]) in float32."""
    g6 = np.asarray(inputs["global_pose_params_6d"], np.float64)
    nk = np.asarray(inputs["neck_pose_params_ax"], np.float64)
    jw = np.asarray(inputs["jaw_pose_params_ax"], np.float64)
    ey = np.asarray(inputs["eye_pose_params_ax"], np.float64)
    jt = np.asarray(inputs["J_transformed_rest"], np.float64)  # [B,5,3]

    Rg = _rot6d(g6)
    Rn = _rodrigues(nk)
    Rj = _rodrigues(jw)
    Rel = _rodrigues(ey[:, :3])
    Rer = _rodrigues(ey[:, 3:])
    rot_mats = np.stack([Rg, Rn, Rj, Rel, Rer], axis=1)  # [B,5,3,3]

    rel = jt.copy()
    rel[:, 1:] -= jt[:, PARENTS[1:]]
    Tm = _make_T(rot_mats, rel)  # [B,5,4,4]
    chain = [Tm[:, 0]]
    for i in range(1, J):
        chain.append(chain[int(PARENTS[i])] @ Tm[:, i])
    tr = np.stack(chain, axis=1)  # [B,5,4,4]
    posed = tr[:, :, :3, 3]
    Rw = tr[:, :, :3, :3]
    t = posed - np.einsum("bjhw,bjw->bjh", Rw, jt)
    A = _make_T(Rw, t)  # [B,5,4,4]

    A34 = np.ascontiguousarray(A[:, :, :3, :4], np.float32)
    PF = np.ascontiguousarray(
        (rot_mats[:, 1:5] - np.eye(3)).reshape(B, -1), np.float32
    )
    return A34, PF


def host_reference_emulation(inputs):
    """Numpy emulation of what the device computes (for validation)."""
    A34, PF = host_prep(inputs)
    vs = np.asarray(inputs["v_shaped_expressed"], np.float32)
    W = np.asarray(inputs["lbs_weights"], np.float32)  # [V,5]
    pd = np.asarray(inputs["posedirs"], np.float32)  # [V,36,3]
    pbs = np.einsum("bp,vpc->bvc", PF, pd)
    v = vs + pbs
    T = np.einsum("bjhw,vj->bvhw", A34, W)  # [B,V,3,4]
    out = np.einsum("bvhw,bvw->bvh", T[:, :, :, :3], v) + T[:, :, :, 3]
    return out.astype(np.float32)


# ---------------------------------------------------------------- bass build


def build_nc(bc=BC):
    import concourse.bacc as bacc
    import concourse.bass as bass_mod
    import concourse.tile as tile
    from concourse import mybir

    f16 = mybir.dt.float16
    f32 = mybir.dt.float32

    nc = bacc.Bacc()
    vs_d = nc.dram_tensor("vs", [bc, 3 * VP], f16, kind="ExternalInput")
    pd_d = nc.dram_tensor("pd", [36, 3 * VP], f16, kind="ExternalInput")
    w2_d = nc.dram_tensor("w2", [10, 2 * VP], f16, kind="ExternalInput")
    pf_d = nc.dram_tensor("pf", [36, bc], f16, kind="ExternalInput")
    a2_d = nc.dram_tensor("a2", [10, 6 * bc], f16, kind="ExternalInput")
    i_d = nc.dram_tensor("ident", [bc, bc], f16, kind="ExternalInput")
    out_d = nc.dram_tensor("out", [bc, 3 * VP], f16, kind="ExternalOutput")

    with tile.TileContext(nc) as tc, ExitStack() as ctx:
        singles = ctx.enter_context(tc.tile_pool(name="singles", bufs=1))
        # small tensors first so chunk 0 can start quickly
        sb_pf = singles.tile([36, bc], f16)
        nc.sync.dma_start(out=sb_pf, in_=pf_d[:])
        sb_i = singles.tile([bc, bc], f16)
        nc.sync.dma_start(out=sb_i, in_=i_d[:])
        sb_a2 = singles.tile([10, 6, bc], f16)
        nc.sync.dma_start(out=sb_a2, in_=a2_d[:])
        sb_pd = singles.tile([36, 3, VP], f16)
        for c in range(3):
            nc.sync.dma_start(
                out=sb_pd[:, c, :], in_=pd_d[:, c * VP : (c + 1) * VP]
            )
        sb_w2 = singles.tile([10, 2, VP], f16)
        nc.sync.dma_start(out=sb_w2, in_=w2_d[:])
        sb_vs = singles.tile([bc, 3, VP], f16)
        for s0 in range(0, VP, VS_SLAB):
            nc.sync.dma_start(
                out=sb_vs[:, :, s0 : s0 + VS_SLAB],
                in_=vs_d[:].rearrange("p (c v) -> p c v", c=3)[
                    :, :, s0 : s0 + VS_SLAB
                ],
            )
        sb_out = singles.tile([bc, 3, VP], f16)

        v_psum = ctx.enter_context(
            tc.tile_pool(name="vpsum", bufs=2, space="PSUM")
        )
        t_psum = ctx.enter_context(
            tc.tile_pool(name="tpsum", bufs=2, space="PSUM")
        )
        t_pool = ctx.enter_context(tc.tile_pool(name="tsb", bufs=2))
        v_pool = ctx.enter_context(tc.tile_pool(name="vsb", bufs=2))
        m_pool = ctx.enter_context(tc.tile_pool(name="msb", bufs=2))
        s_pool = ctx.enter_context(tc.tile_pool(name="ssb", bufs=2))

        for ci in range(NCHUNK):
            c0 = ci * CH
            cs = slice(c0, c0 + CH)

            # ---- v = PF @ posedirs + vs, accumulated in PSUM (fp32)
            vp = v_psum.tile([bc, 4, CH], f32, tag="vp")  # planes 0..2 used
            nc.tensor.matmul(
                vp[:, 0:2, :], lhsT=sb_pf, rhs=sb_pd[:, 0:2, cs],
                start=True, stop=False,
            )
            nc.tensor.matmul(
                vp[:, 2, :], lhsT=sb_pf, rhs=sb_pd[:, 2, cs],
                start=True, stop=False,
            )
            nc.tensor.matmul(
                vp[:, 0:2, :], lhsT=sb_i, rhs=sb_vs[:, 0:2, cs],
                start=False, stop=True,
            )
            nc.tensor.matmul(
                vp[:, 2, :], lhsT=sb_i, rhs=sb_vs[:, 2, cs],
                start=False, stop=True,
            )

            # DVE drains v (psum f32 -> sbuf f16)
            v_sb = v_pool.tile([bc, 3, CH], f16, tag="v")
            nc.vector.tensor_copy(v_sb[:], vp[:, 0:3, :])

            # ---- T[h] = blockdiag(W2) @ A2[h,p], two w-planes per matmul
            t_sb = t_pool.tile([bc, 3, 4, CH], f16, tag="t")
            for h in range(3):
                tp = t_psum.tile([bc, 4, CH], f32, tag="tp")
                for p in range(2):
                    nc.tensor.matmul(
                        tp[:, 2 * p : 2 * p + 2, :],
                        lhsT=sb_a2[:, 2 * h + p, :],
                        rhs=sb_w2[:, :, cs],
                        start=True, stop=True,
                    )
                # Act drains T[h] (1024 cols)
                nc.scalar.copy(t_sb[:, h, :, :], tp[:])

            # ---- combine: out_h = sum_w T_hw*v_w + T_h3  (fp16, 2x mode)
            vs_ap = v_sb[:]
            vb = bass_mod.AP(  # [bc, 3h(bcast), 3w, CH] stride-0 on h
                tensor=vs_ap.tensor,
                offset=vs_ap.offset,
                ap=[list(vs_ap.ap[0]), [0, 3], [CH, 3], [1, CH]],
            )
            m = m_pool.tile([bc, 3, 3, CH], f16, tag="m")
            nc.vector.tensor_tensor(
                m[:], t_sb[:, :, 0:3, :], vb, op=mybir.AluOpType.mult
            )
            s1 = s_pool.tile([bc, 3, CH], f16, tag="s1")
            nc.vector.tensor_add(s1[:], m[:, :, 0, :], m[:, :, 1, :])
            s2 = s_pool.tile([bc, 3, CH], f16, tag="s2")
            nc.gpsimd.tensor_tensor(
                s2[:], s1[:], m[:, :, 2, :], op=mybir.AluOpType.add
            )
            nc.vector.tensor_add(sb_out[:, :, cs], s2[:], t_sb[:, :, 3, :])

        for s0 in range(0, VP, VS_SLAB):
            nc.sync.dma_start(
                out=out_d[:].rearrange("p (c v) -> p c v", c=3)[
                    :, :, s0 : s0 + VS_SLAB
                ],
                in_=sb_out[:, :, s0 : s0 + VS_SLAB],
            )

    _strip_matmul_self_waits(nc)
    if not nc.is_finalized():
        nc.finalize()
    return nc


def _strip_matmul_self_waits(nc):
    """Drop redundant same-engine self-waits from Matmult instructions.

    Tile emits pool-slot release waits for every accessor proc, including the
    PE itself. With a fully unrolled kernel the PE queue executes in order, so
    a PE instruction waiting on the PE tick semaphore is always already
    satisfied — but walrus codegen only has one sync-wait slot for LDWEIGHTS,
    so a matmul carrying [other-engine wait, PE self-wait] fails to compile.
    """
    fn = nc.m.functions[0]
    pe_sems = set()
    for b in fn.blocks:
        for i in b.instructions:
            if i.opcode == "Matmult":
                for u in i.sync_info.on_update:
                    if u.ant_name.startswith("PE"):
                        pe_sems.add(u.ant_name)
    for b in fn.blocks:
        for i in b.instructions:
            if i.opcode != "Matmult":
                continue
            si = i.sync_info
            kept = [w for w in si.on_wait if w.ant_name not in pe_sems]
            if len(kept) != len(si.on_wait):
                si.on_wait = kept
                i.sync_info = si


# ---------------------------------------------------------------- entry point

_BUILT = {}


def _get_nc():
    if "nc" not in _BUILT:
        _BUILT["nc"] = build_nc()
    return _BUILT["nc"]


def make_in_maps(inputs):
    A34, PF = host_prep(inputs)
    vs = np.asarray(inputs["v_shaped_expressed"], np.float32)  # [B,V,3]
    W = np.asarray(inputs["lbs_weights"], np.float32)  # [V,5]
    pd = np.asarray(inputs["posedirs"], np.float32)  # [V,36,3]

    # vs, w-major + padded: [B, 3, VP]
    vs_wm = np.zeros((B, 3, VP), F16)
    vs_wm[:, :, :V] = vs.transpose(0, 2, 1)

    # posedirs w-major: pd_wm[p, c, v]
    pd_wm = np.zeros((36, 3, VP), F16)
    pd_wm[:, :, :V] = pd.transpose(1, 2, 0)
    pd_flat = np.ascontiguousarray(pd_wm.reshape(36, 3 * VP))

    # W2 block-diag over w-pairs: W2[j*2+wi, wi', v] = W[v,j] if wi==wi'
    W2 = np.zeros((10, 2, VP), F16)
    Wt = W.T.astype(F16)  # [5, V]
    for j in range(5):
        W2[j * 2 + 0, 0, :V] = Wt[j]
        W2[j * 2 + 1, 1, :V] = Wt[j]
    w2_flat = np.ascontiguousarray(W2.reshape(10, 2 * VP))

    ident = np.eye(BC, dtype=F16)

    PFt = PF.T.astype(F16)  # [36, B]

    in_maps = []
    for c in range(NCORES):
        sl = slice(c * BC, (c + 1) * BC)
        # A2[j*2+wi, (h*2+p)*BC + b] = A34[b, j, h, 2p+wi]
        A_c = A34[sl]  # [bc, 5, 3, 4]
        a2 = np.empty((10, 6, BC), np.float32)
        for h in range(3):
            for p in range(2):
                # [bc, 5, 2] -> [5, 2, bc] -> [10, bc]
                blk = A_c[:, :, h, 2 * p : 2 * p + 2].transpose(1, 2, 0)
                a2[:, h * 2 + p, :] = blk.reshape(10, BC)
        in_maps.append(
            {
                "vs": np.ascontiguousarray(vs_wm[sl].reshape(BC, 3 * VP)),
                "pd": pd_flat,
                "w2": w2_flat,
                "pf": np.ascontiguousarray(PFt[:, sl]),
                "a2": np.ascontiguousarray(a2.reshape(10, 6 * BC).astype(F16)),
                "ident": ident,
                "out": None,
            }
        )
        del in_maps[-1]["out"]
    return in_maps


def run_on_device(inputs, trace=False):
    from concourse.bass_utils import run_bass_kernel_spmd

    nc = _get_nc()
    in_maps = make_in_maps(inputs)
    res = run_bass_kernel_spmd(nc, in_maps, list(range(NCORES)), trace=trace)
    out = np.concatenate(
        [res.results[i]["out"].reshape(BC, 3, VP) for i in range(NCORES)],
        axis=0,
    )  # [B, 3, VP] f16
    out = out[:, :, :V].transpose(0, 2, 1)  # [B, V, 3]
    return np.ascontiguousarray(out, np.float32), res


def kernel(**inputs):
    out, _ = run_on_device(inputs, trace=False)
    return out
